# revision 12
# baseline (speedup 1.0000x reference)
"""ContextualLoss on 8 Trainium2 NeuronCores (Bass/Tile).

Problem: nn_ContextualLoss — N=4, C=64, H=W=64, P=H*W=4096.

Math (per batch n):
  meanT    = mean of T over (N,H,W)                              [C]
  Tc/Ic    = centered features;  h_p = 1/|Tc_p|, g_q = 1/|Ic_q|
  cos[q,p] = (Ic_q . Tc_p) * h_p                                 [P, P]
  mq       = max_p cos ; a2 = 1/(1+2eps - g*mq); sc = a2*g; bias = 1-a2
  cs_w     = exp(sc*cos + bias); cs = cs_w / sum_p cs_w
  k_p      = max_q cs ; CS_n = mean_p k_p ; score = mean_n(-log CS_n)

Sharding: 2 cores per batch; each core owns 2048 q rows (all 4096 p cols),
so row max/sum are core-local. Core outputs partial column-max k [128, P]
(partition i = max over its 16 q-blocks); host reduces across cores/blocks.
The host rotates batches per core so each core's batch T is always flat
rows 0-63 of the packed t_full layout — one SPMD program for all cores,
and no separate t_own transfer (tcent is computed from a t_full slice).

Main loop per 128-q block: PE matmuls (fp32r) fill two 4-bank PSUM tiles
[128,2048] with cos' (h pre-folded into tn); one DVE reduce_max per tile;
ACT exp reads PSUM directly (scale/bias per-partition, fused row-sum
accum), writing bf16 eb — no PSUM->SBUF copy pass at all (the baseline
spent a full ACT pass on it); DVE normalizes (4x bf16) and
max-accumulates into k (2x bf16).

Prologue: meanT accumulated behind the t_full DMA; h = exp(-0.5 ln sumsq)
computed in block-compact [128,32] layout (ACT cost 32 cols, not 4096),
flattened via 4 32x32 stream transposes + 1 DMA, broadcast to 64
partitions by doubling DMAs; g likewise in [128,16] layout.
"""

import numpy as np

import concourse.bacc as bacc_mod
import concourse.mybir as mybir
import concourse.tile as tile
from concourse.bass_utils import run_bass_kernel_spmd

N, C, H, W = 4, 64, 64, 64
P = H * W                  # 4096 template pixels
QH = P // 2                # 2048 query pixels per core
NBLK = QH // 128           # 16 q-blocks per core
NCORES = 8
EPS = 1e-5
F32 = mybir.dt.float32
BF16 = mybir.dt.bfloat16
F32R = mybir.dt.float32r
AX = mybir.AxisListType
OP = mybir.AluOpType
AF = mybir.ActivationFunctionType

MM_DT = F32R       # matmul input dtype
E_DT = BF16        # eb / cs / k dtype


def build_nc():
    nc = bacc_mod.Bacc("TRN2", target_bir_lowering=False, debug=False)

    t_full = nc.dram_tensor("t_full", [128, 2 * P], F32, kind="ExternalInput")
    i_own = nc.dram_tensor("i_own", [C, QH], F32, kind="ExternalInput")
    k_out = nc.dram_tensor("k_out", [128, P], E_DT, kind="ExternalOutput")

    with tile.TileContext(nc) as tc:
        with (
            tc.tile_pool(name="persist", bufs=1) as pp,
            tc.tile_pool(name="small", bufs=4) as sp,
        ):
            # ---------------- persistent tiles ----------------
            tf = pp.tile([128, 2 * P], F32)    # raw T, packed rows (p, p+128)
            tn = pp.tile([C, P], MM_DT)        # centered+h-scaled T (rhs)
            ic = pp.tile([C, QH], MM_DT)       # centered I (lhsT)
            ktile = pp.tile([128, P], E_DT)    # running column max
            g = pp.tile([128, NBLK], F32)      # 1/|Ic_q| in block layout
            negg = pp.tile([128, NBLK], F32)   # -g
            onecp = pp.tile([128, 1], F32)     # 1 + 2*eps
            ones64f = pp.tile([C, 1], F32)

            nc.vector.memset(ktile, 0.0)
            nc.vector.memset(onecp, 1.0 + 2.0 * EPS)
            nc.vector.memset(ones64f, 1.0)

            # ---------------- prologue ----------------
            with (
                tc.tile_pool(name="pro", bufs=1) as pro,
                tc.tile_pool(name="pps", bufs=1, space="PSUM") as pps,
            ):
                # meanT: partition p holds flat T rows p and 128+p
                # (row r = n*64 + c); 4 chunks pipeline accum behind DMA.
                macc4 = sp.tile([128, 4], F32)
                tsc = pro.tile([128, 2048], BF16, tag="tsc")
                for j in range(4):
                    nc.sync.dma_start(out=tf[:, j * 2048:(j + 1) * 2048],
                                      in_=t_full[:, j * 2048:(j + 1) * 2048])
                    nc.scalar.activation(out=tsc, in_=tf[:, j * 2048:
                                                         (j + 1) * 2048],
                                         func=AF.Copy,
                                         accum_out=macc4[:, j:j + 1])
                macc = sp.tile([128, 1], F32)
                nc.vector.reduce_sum(out=macc, in_=macc4, axis=AX.X)
                ms = sp.tile([128, 1], F32)
                nc.vector.tensor_scalar_mul(ms, macc, -1.0 / (N * P))
                rot0 = sp.tile([C, 1], F32)
                nc.sync.dma_start(out=rot0, in_=ms[64:128, :])
                negmu = sp.tile([C, 1], F32, tag="negmu")
                nc.vector.tensor_tensor(out=negmu, in0=ms[0:C, 0:1],
                                        in1=rot0, op=OP.add)

                iown = pro.tile([C, QH], F32)
                nc.sync.dma_start(out=iown, in_=i_own[:, :])
                # center I on ACT (bias = -mu per partition), f32r out
                nc.scalar.activation(out=ic, in_=iown, func=AF.Identity,
                                     bias=negmu, scale=1.0)
                sqi = pro.tile([C, QH], F32, tag="sqi")
                nc.scalar.square(sqi, ic)

                # center this core's T (always tf rows 0-63, cols 0:P after
                # the host-side batch rotation); squares on DVE
                tcent = pro.tile([C, P], F32, tag="tcent")
                sqt = pro.tile([C, P], F32, tag="sqt")
                for cch in range(8):
                    sl = slice(cch * 512, (cch + 1) * 512)
                    nc.scalar.activation(out=tcent[:, sl], in_=tf[0:64, sl],
                                         func=AF.Identity, bias=negmu,
                                         scale=1.0)
                    nc.vector.tensor_tensor(out=sqt[:, sl], in0=tcent[:, sl],
                                            in1=tcent[:, sl], op=OP.mult)

                # sumsq per template pixel, block-compact [128, 32]:
                # 32 K=64 matmuls (lhsT = sqt 128-col slice, rhs = ones)
                sqb = pps.tile([128, 32], F32, tag="sqb")
                for b in range(32):
                    nc.tensor.matmul(sqb[:, b:b + 1],
                                     sqt[:, b * 128:(b + 1) * 128],
                                     ones64f, start=True, stop=True)
                # h = exp(-0.5*ln(sumsq)) in compact layout
                lnb = sp.tile([128, 32], F32, tag="lnb")
                nc.scalar.activation(out=lnb, in_=sqb, func=AF.Ln)
                hblk = sp.tile([128, 32], F32, tag="hblk")
                nc.scalar.activation(out=hblk, in_=lnb, func=AF.Exp,
                                     scale=-0.5)
                # transpose to [32, 128] so the flatten to [1, 4096] is
                # plain partition-major order
                hblkT = sp.tile([32, 128], F32, tag="hblkT")
                for j in range(4):
                    nc.vector.transpose(hblkT[:, 32 * j:32 * (j + 1)],
                                        hblk[32 * j:32 * (j + 1), :])
                ht = pro.tile([1, P], F32, tag="ht")
                nc.sync.dma_start(out=ht, in_=hblkT)
                # broadcast to 64 partitions by doubling
                hbc = pro.tile([C, P], F32, tag="hbc")
                nc.sync.dma_start(out=hbc[0:1, :], in_=ht)
                pc = 1
                while pc < C:
                    nc.sync.dma_start(out=hbc[pc:2 * pc, :],
                                      in_=hbc[0:pc, :])
                    pc *= 2
                # tn = tcent * h  (fold h into the matmul rhs), f32r out
                for cch in range(8):
                    sl = slice(cch * 512, (cch + 1) * 512)
                    nc.vector.tensor_tensor(out=tn[:, sl], in0=tcent[:, sl],
                                            in1=hbc[:, sl], op=OP.mult)

                # g = 1/|Ic_q| block layout via 16 tiny matmuls -> [128, 16]
                g2 = pps.tile([128, NBLK], F32, tag="g2")
                for b in range(NBLK):
                    nc.tensor.matmul(g2[:, b:b + 1],
                                     sqi[:, b * 128:(b + 1) * 128],
                                     ones64f, start=True, stop=True)
                lng = sp.tile([128, NBLK], F32, tag="lng")
                nc.scalar.activation(out=lng, in_=g2, func=AF.Ln)
                nc.scalar.activation(out=g, in_=lng, func=AF.Exp, scale=-0.5)
                nc.vector.tensor_scalar_mul(negg, g, -1.0)

            # ---------------- main loop ----------------
            HW_ = P // 2
            with (
                tc.tile_pool(name="ebuf", bufs=2) as ep,
                tc.tile_pool(name="csb", bufs=2) as csp,
                tc.tile_pool(name="mps", bufs=1, space="PSUM") as mps,
            ):
                for b in range(NBLK):
                    lhs = ic[:, b * 128:(b + 1) * 128]
                    rm2 = sp.tile([128, 2], F32, tag="rm2")
                    eb = ep.tile([128, P], E_DT, tag="eb")
                    ss2 = sp.tile([128, 2], F32, tag="ss2")
                    pss = []
                    for h in range(2):
                        ps = mps.tile([128, HW_], F32, tag=f"ps{h}")
                        pss.append(ps)
                        for cch in range(HW_ // 512):
                            off = h * HW_ + cch * 512
                            nc.tensor.matmul(
                                ps[:, cch * 512:(cch + 1) * 512], lhs,
                                tn[:, off:off + 512],
                                start=True, stop=True)
                        nc.vector.reduce_max(out=rm2[:, h:h + 1], in_=ps,
                                             axis=AX.X)
                    mq = sp.tile([128, 1], F32, tag="mq")
                    nc.vector.reduce_max(out=mq, in_=rm2, axis=AX.X)
                    dd = sp.tile([128, 1], F32, tag="dd")
                    nc.vector.scalar_tensor_tensor(
                        out=dd, in0=mq, scalar=negg[:, b:b + 1], in1=onecp,
                        op0=OP.mult, op1=OP.add)
                    a2 = sp.tile([128, 1], F32, tag="a2")
                    nc.vector.reciprocal(a2, dd)
                    sc = sp.tile([128, 1], F32, tag="sc")
                    nc.vector.tensor_tensor(out=sc, in0=a2, in1=g[:, b:b + 1],
                                            op=OP.mult)
                    bias = sp.tile([128, 1], F32, tag="bias")
                    nc.vector.tensor_scalar(out=bias, in0=a2, scalar1=-1.0,
                                            scalar2=1.0, op0=OP.mult,
                                            op1=OP.add)
                    for h in range(2):
                        nc.scalar.activation(
                            out=eb[:, h * HW_:(h + 1) * HW_], in_=pss[h],
                            func=AF.Exp, bias=bias, scale=sc,
                            accum_out=ss2[:, h:h + 1])
                    ssum = sp.tile([128, 1], F32, tag="ssum")
                    nc.vector.tensor_tensor(out=ssum, in0=ss2[:, 0:1],
                                            in1=ss2[:, 1:2], op=OP.add)
                    rr = sp.tile([128, 1], F32, tag="rr")
                    nc.vector.reciprocal(rr, ssum)
                    cs = csp.tile([128, P], E_DT, tag="cs")
                    nc.vector.tensor_scalar(out=cs, in0=eb, scalar1=rr,
                                            scalar2=None, op0=OP.mult)
                    nc.vector.tensor_tensor(out=ktile, in0=ktile, in1=cs,
                                            op=OP.max)

            nc.sync.dma_start(out=k_out[:, :], in_=ktile)

    nc.compile()
    return nc


_NC_CACHE = {}


def _get_nc():
    if "nc" not in _NC_CACHE:
        _NC_CACHE["nc"] = build_nc()
    return _NC_CACHE["nc"]


def make_in_maps(I_features, T_features):
    I4 = np.ascontiguousarray(
        np.asarray(I_features, dtype=np.float32).reshape(N, C, P))
    T4 = np.ascontiguousarray(
        np.asarray(T_features, dtype=np.float32).reshape(N, C, P))
    in_maps = []
    for core in range(NCORES):
        n, half = core // 2, core % 2
        # rotate batches so this core's batch is flat rows 0-63; meanT is
        # order-invariant. partition p holds flat rows p and p+128.
        perm = [(n + j) % N for j in range(N)]
        tf = np.ascontiguousarray(
            T4[perm].reshape(2, 128, P).transpose(1, 0, 2).reshape(128, 2 * P))
        in_maps.append({
            "t_full": tf,
            "i_own": np.ascontiguousarray(I4[n][:, half * QH:(half + 1) * QH]),
        })
    return in_maps


def finish_host(kparts):
    """kparts: [8, 128, P] per-core partial column maxima -> scalar score."""
    ks = np.stack([np.asarray(kp, dtype=np.float64) for kp in kparts])
    kp = ks.reshape(N, 2 * 128, P).max(axis=1)      # [N, P]
    cs = kp.mean(axis=1)                            # [N]
    return np.float32(np.mean(-np.log(cs)))


def kernel(I_features, T_features, _trace=False):
    nc = _get_nc()
    in_maps = make_in_maps(I_features, T_features)
    res = run_bass_kernel_spmd(nc, in_maps, core_ids=list(range(NCORES)),
                               trace=_trace)
    score = finish_host([r["k_out"] for r in res.results])
    if _trace:
        return np.array(score, dtype=np.float32), res
    return np.array(score, dtype=np.float32)
